# revision 30
# baseline (speedup 1.0000x reference)
"""MoLA adapter (MoE-of-LoRA) Trainium2 kernel.

out = x @ W_base.T + b_base
      + sum_{e in top2(router(x))} softmax_gate_e * (x @ A_e.T @ B_e.T) * (alpha/r)

Strategy (8 NeuronCores, data-parallel over tokens; 1024 tokens/core):
  - Host pre-transposes x (-> xT [D, 1024] per core) and W_base (-> wT [D, O]),
    packs LoRA-A as rc [D, 64] (f32r), router as rcl [D, 8] (f32), and
    (SCALE*B_cat | b_base) as bc [65, O] (f32r).
  - hT [64+1, tok]: A.T @ x with A stationary, x moving at N=512 (f32r full
    rate); row 64 is ones (carries the bias through the combine matmul).
  - logits: token-major [128, 8] per tile via N=8 fp32 matmuls (exact top-k
    selection; f32r logits flip near-tie experts), 8 groups into one PSUM bank.
  - top-2 + softmax-of-2 gates: one batched DVE/ACT chain over all 8 token
    tiles at once ([128, 8 tiles x 8 experts]), then per tile a broadcast
    gate-replicate [128, 65], PE transpose, and whT = hT * gatesT (f32->f32r).
  - y tile [128, 512] PSUM accumulates 16 base matmuls (xT_k.T @ wT_k, f32r,
    1 col/cycle) plus one K=65 combine matmul whT.T @ bc (LoRA update + bias).
  - float32r is the full-rate fp32 PE path (1 col/cycle at N>=256 vs 4 for
    fp32): measured l2 rel err 1.5e-4 on a K=2048 matmul, 16x better than
    bf16, at bf16 speed.
  - DMA: ~33 MB/core at ~360 GB/s is ~92 us vs ~127 us PE busy. Loads go on
    the SP HWDGE ring in 0.5-2 MB chunks (x staggered small-first so the PE
    starts at ~2 us; W0 quarters interleaved so base groups chase the
    stream); stores and small constants go on the ACT HWDGE ring. 5 PSUM
    banks of open base-accumulation groups hide most of the x+W0 prefix.
"""

import numpy as np

import concourse.bass as bass
import concourse.mybir as mybir
import concourse.tile as tile
from concourse import bacc
from concourse.bass_utils import run_bass_kernel_spmd
from concourse.masks import make_identity

# problem dims (hardcoded per contract)
B, S, D, O = 4, 2048, 2048, 2048
E, R, TOPK = 8, 8, 2
SCALE = 16.0 / R  # alpha / r
NCORES = 8
NTOK = B * S
TOK = NTOK // NCORES        # tokens per core = 1024
NT = TOK // 128             # 8 token tiles per core
KC = D // 128               # 16 contraction chunks
OCH = 512
NOC = O // OCH              # 4 output chunks
ER = E * R                  # 64
RC = E + ER                 # 72: router logits + all-expert h

F32 = mybir.dt.float32
F32R = mybir.dt.float32r

_CACHE = {}


def _build_program(use_f32r=True):
    key = ("prog", use_f32r)
    if key in _CACHE:
        return _CACHE[key]
    nc = bacc.Bacc("TRN2", target_bir_lowering=False, debug=False)
    dt_mm = F32R if use_f32r else F32
    TC2 = TOK // 512            # 2 chunks of 512 tokens for the prT matmul
    xT = nc.dram_tensor("xT", [D, TOK], dt_mm, kind="ExternalInput")
    wT = nc.dram_tensor("wT", [D, O], dt_mm, kind="ExternalInput")
    rc = nc.dram_tensor("rc", [D, ER], dt_mm, kind="ExternalInput")
    rcl = nc.dram_tensor("rcl", [D, E], F32, kind="ExternalInput")
    bc = nc.dram_tensor("bc", [ER + 1, O], dt_mm, kind="ExternalInput")
    y = nc.dram_tensor("y", [TOK, O], F32, kind="ExternalOutput")

    with tile.TileContext(nc) as tc:
        with (
            tc.tile_pool(name="const", bufs=1) as const,
            tc.tile_pool(name="xpool", bufs=1) as xpool,
            tc.tile_pool(name="wpool", bufs=2) as wpool,
            tc.tile_pool(name="small", bufs=2) as small,
            tc.tile_pool(name="whtp", bufs=1) as whtp,
            tc.tile_pool(name="opool", bufs=4) as opool,
            tc.tile_pool(name="pso", bufs=5, space="PSUM") as pso,
            tc.tile_pool(name="psrt", bufs=1, space="PSUM") as psrt,
            tc.tile_pool(name="psaux", bufs=2, space="PSUM") as psaux,
        ):
            ident = const.tile([128, 128], F32)
            make_identity(nc, ident[:])

            # router+A params, transposed-stationary: rc_sb[p, k*RC+c]
            rc_sb = const.tile([128, KC * ER], dt_mm)
            nc.scalar.dma_start(
                out=rc_sb[:].rearrange("p (k c) -> p k c", k=KC),
                in_=rc[:, :].rearrange("(k p) c -> p k c", p=128),
            )
            rcl_sb = const.tile([128, KC * E], F32)
            nc.scalar.dma_start(
                out=rcl_sb[:].rearrange("p (k c) -> p k c", k=KC),
                in_=rcl[:, :].rearrange("(k p) c -> p k c", p=128),
            )

            # x resident: one big tile, xbig[p, k*TOK + tok] = xT[k*128+p, tok]
            # W for oc0 interleaved so pass-2 oc0 can start while x streams.
            xbig = xpool.tile([128, KC * TOK], dt_mm, name="xbig")
            KQ = KC // 4  # 4 k-blocks per W DMA
            # x k-block DMA sizes: small first so prT matmuls start early
            xsizes = [1, 1, 2, 4, 4, 4]
            w0_after = {4: 0, 8: 1, 12: 2, 16: 3}  # k0 -> w0 quarter to issue
            w0ks = []
            k0 = 0
            for sz in xsizes:
                nc.sync.dma_start(
                    out=xbig[:, k0 * TOK:(k0 + sz) * TOK].rearrange(
                        "p (k c) -> p k c", k=sz
                    ),
                    in_=xT[k0 * 128:(k0 + sz) * 128, :].rearrange(
                        "(k p) c -> p k c", p=128
                    ),
                )
                k0 += sz
                if k0 in w0_after:
                    q = w0_after[k0]
                    wq = wpool.tile(
                        [128, KQ * OCH], dt_mm, tag=f"w{q}",
                        bufs=3, name=f"w0_{q}"
                    )
                    nc.sync.dma_start(
                        out=wq[:].rearrange("p (k c) -> p k c", k=KQ),
                        in_=wT[q * KQ * 128:(q + 1) * KQ * 128, 0:OCH].rearrange(
                            "(k p) c -> p k c", p=128
                        ),
                    )
                    w0ks.append(wq)

            bc_sb = const.tile([ER + 1, O], dt_mm)
            nc.scalar.dma_start(out=bc_sb[:], in_=bc[:, :])

            # queue the rest of W now: deep DMA lookahead keeps the SP ring busy
            wks_all = [w0ks]
            for oc in range(1, NOC):
                wks = []
                for q in range(4):
                    wq = wpool.tile(
                        [128, KQ * OCH], dt_mm, tag=f"w{q}",
                        bufs=3, name=f"w{oc}_{q}"
                    )
                    nc.sync.dma_start(
                        out=wq[:].rearrange("p (k c) -> p k c", k=KQ),
                        in_=wT[q * KQ * 128:(q + 1) * KQ * 128,
                               oc * OCH:(oc + 1) * OCH].rearrange(
                            "(k p) c -> p k c", p=128
                        ),
                    )
                    wks.append(wq)
                wks_all.append(wks)

            # ---- pass 1a: hT = A.T @ x -> [64, 512] per chunk (f32r),
            #      logits token-major in f32 (exact top-k)
            prTs = []
            for tcn in range(TC2):
                prT = psrt.tile([ER, 512], F32, tag="prT", name=f"prT{tcn}")
                for k in range(KC):
                    nc.tensor.matmul(
                        prT[:],
                        rc_sb[:, k * ER:(k + 1) * ER],
                        xbig[:, k * TOK + tcn * 512:k * TOK + tcn * 512 + 512],
                        start=(k == 0),
                        stop=(k == KC - 1),
                    )
                prTs.append(prT)

            # hT in sbuf (f32): rows e*8+j, cols = token; row 64 = ones (bias)
            hT = const.tile([ER + 1, TOK], F32)
            nc.vector.memset(hT[ER:ER + 1, :], 1.0)
            for tcn in range(TC2):
                nc.vector.tensor_copy(
                    hT[0:ER, tcn * 512:(tcn + 1) * 512], prTs[tcn][0:ER, :]
                )

            # logits: 8 groups (one per token tile) into one PSUM bank, f32
            plg = psaux.tile([128, NT * E], F32, tag="aux", name="plg")
            for t in range(NT):
                for k in range(KC):
                    nc.tensor.matmul(
                        plg[:, t * E:(t + 1) * E],
                        xbig[:, k * TOK + t * 128:k * TOK + t * 128 + 128]
                        .bitcast(F32),
                        rcl_sb[:, k * E:(k + 1) * E],
                        start=(k == 0),
                        stop=(k == KC - 1),
                    )
            LG = small.tile([128, NT * E], F32, tag="LG", name="LG")
            nc.vector.tensor_copy(LG[:], plg[:])

            # ---- pass 1b: batched top-2 softmax gates over all NT tiles ----
            LG3 = LG[:].rearrange("p (t e) -> p t e", t=NT)
            m1 = small.tile([128, NT], F32, tag="m1", name="m1")
            nc.vector.reduce_max(m1[:], LG3, axis=mybir.AxisListType.X)
            selmax = small.tile([128, NT * E], F32, tag="selmax", name="selmax")
            nc.vector.tensor_tensor(
                out=selmax[:].rearrange("p (t e) -> p t e", t=NT),
                in0=LG3,
                in1=m1[:].unsqueeze(-1).broadcast_to([128, NT, E]),
                op=mybir.AluOpType.is_ge,
            )
            masked = small.tile([128, NT * E], F32, tag="masked", name="masked")
            nc.vector.scalar_tensor_tensor(
                out=masked[:], in0=selmax[:], scalar=-1e30, in1=LG[:],
                op0=mybir.AluOpType.mult, op1=mybir.AluOpType.add,
            )
            m2 = small.tile([128, NT], F32, tag="m2", name="m2")
            nc.vector.reduce_max(
                m2[:], masked[:].rearrange("p (t e) -> p t e", t=NT),
                axis=mybir.AxisListType.X,
            )
            d1 = small.tile([128, NT * E], F32, tag="d1", name="d1")
            nc.vector.tensor_tensor(
                out=d1[:].rearrange("p (t e) -> p t e", t=NT),
                in0=LG3,
                in1=m1[:].unsqueeze(-1).broadcast_to([128, NT, E]),
                op=mybir.AluOpType.subtract,
            )
            eall = small.tile([128, NT * E], F32, tag="eall", name="eall")
            nc.scalar.activation(
                eall[:], d1[:], mybir.ActivationFunctionType.Exp,
            )
            d2 = small.tile([128, NT], F32, tag="d2", name="d2")
            nc.vector.tensor_sub(d2[:], m2[:], m1[:])
            e2 = small.tile([128, NT], F32, tag="e2", name="e2")
            nc.scalar.activation(
                e2[:], d2[:], mybir.ActivationFunctionType.Exp,
            )
            denom = small.tile([128, NT], F32, tag="denom", name="denom")
            nc.vector.tensor_scalar(
                out=denom[:], in0=e2[:], scalar1=1.0, scalar2=None,
                op0=mybir.AluOpType.add,
            )
            invd = small.tile([128, NT], F32, tag="invd", name="invd")
            nc.vector.reciprocal(invd[:], denom[:])
            sel = small.tile([128, NT * E], F32, tag="sel", name="sel")
            nc.vector.tensor_tensor(
                out=sel[:].rearrange("p (t e) -> p t e", t=NT),
                in0=LG3,
                in1=m2[:].unsqueeze(-1).broadcast_to([128, NT, E]),
                op=mybir.AluOpType.is_ge,
            )
            gsel = small.tile([128, NT * E], F32, tag="gsel", name="gsel")
            nc.vector.tensor_mul(gsel[:], eall[:], sel[:])
            ginv = small.tile([128, NT * E], F32, tag="ginv", name="ginv")
            nc.vector.tensor_tensor(
                out=ginv[:].rearrange("p (t e) -> p t e", t=NT),
                in0=gsel[:].rearrange("p (t e) -> p t e", t=NT),
                in1=invd[:].unsqueeze(-1).broadcast_to([128, NT, E]),
                op=mybir.AluOpType.mult,
            )

            # g_rep[t]: [128, 64] with col e*8+j = gate[tok, e]
            greps = []
            for t in range(NT):
                grep = small.tile(
                    [128, ER + 1], F32, tag=f"grep{t}", name=f"grep{t}"
                )
                nc.vector.tensor_copy(
                    grep[:, 0:ER].rearrange("p (e r) -> p e r", e=E),
                    ginv[:, t * E:(t + 1) * E].unsqueeze(-1).broadcast_to(
                        [128, E, R]
                    ),
                )
                nc.vector.memset(grep[:, ER:ER + 1], 1.0)
                greps.append(grep)
            whts = []
            for half in range(2):
                gtp = psaux.tile(
                    [ER + 1, 4 * 128], F32, tag="aux", name=f"gtp{half}"
                )
                for i in range(4):
                    t = half * 4 + i
                    nc.tensor.transpose(
                        gtp[:, i * 128:(i + 1) * 128], greps[t][:], ident[:]
                    )
                for i in range(4):
                    t = half * 4 + i
                    wht = whtp.tile(
                        [ER + 1, 128], dt_mm, tag=f"wht{t}", name=f"wht{t}"
                    )
                    nc.vector.tensor_mul(
                        wht[:], hT[:, t * 128:(t + 1) * 128],
                        gtp[:, i * 128:(i + 1) * 128],
                    )
                    whts.append(wht)

            # ---- pass 2: base matmul + LoRA combine + bias ----
            KQ = KC // 4
            for oc in range(NOC):
                wks = wks_all[oc]
                for t in range(NT):
                    po = pso.tile([128, OCH], F32, tag="po", name=f"po{oc}_{t}")
                    for k in range(KC):
                        nc.tensor.matmul(
                            po[:],
                            xbig[:, k * TOK + t * 128:k * TOK + t * 128 + 128],
                            wks[k // KQ][:, (k % KQ) * OCH:(k % KQ + 1) * OCH],
                            start=(k == 0),
                            stop=False,
                        )
                    nc.tensor.matmul(
                        po[:],
                        whts[t][:],
                        bc_sb[:, oc * OCH:(oc + 1) * OCH],
                        start=False,
                        stop=True,
                    )
                    ot = opool.tile([128, OCH], F32, tag="ot", name=f"ot{oc}_{t}")
                    if (oc * NT + t) % 2 == 0:
                        nc.vector.tensor_copy(ot[:], po[:])
                    else:
                        nc.scalar.copy(ot[:], po[:])
                    nc.scalar.dma_start(
                        out=y[t * 128:(t + 1) * 128, oc * OCH:(oc + 1) * OCH],
                        in_=ot[:],
                    )
    nc.compile()
    _CACHE[key] = nc
    return nc


def _prep_shared(W_base, b_base, W_router, A_w, B_w):
    wT = np.ascontiguousarray(W_base.T)                       # [D, O]
    rc = np.ascontiguousarray(
        A_w.transpose(2, 0, 1).reshape(D, ER).astype(np.float32)
    )                                                         # [D, 64]
    rcl = np.ascontiguousarray(W_router.T.astype(np.float32))  # [D, 8]
    bc = np.concatenate(
        [SCALE * B_w.transpose(0, 2, 1).reshape(ER, O), b_base[None, :]], axis=0
    ).astype(np.float32)                                      # [65, O]
    return np.ascontiguousarray(wT), rc, rcl, np.ascontiguousarray(bc)


def kernel(x, W_base, b_base, W_router, A_w, B_w, _trace=False):
    x = np.asarray(x, dtype=np.float32)
    W_base = np.asarray(W_base, dtype=np.float32)
    b_base = np.asarray(b_base, dtype=np.float32)
    W_router = np.asarray(W_router, dtype=np.float32)
    A_w = np.asarray(A_w, dtype=np.float32)
    B_w = np.asarray(B_w, dtype=np.float32)

    nc = _build_program()
    wT, rc, rcl, bc = _prep_shared(W_base, b_base, W_router, A_w, B_w)
    x_flat = x.reshape(NTOK, D)
    in_maps = []
    for i in range(NCORES):
        shard = x_flat[i * TOK:(i + 1) * TOK]
        in_maps.append({
            "xT": np.ascontiguousarray(shard.T),
            "wT": wT, "rc": rc, "rcl": rcl, "bc": bc,
        })
    res = run_bass_kernel_spmd(
        nc, in_maps, core_ids=list(range(NCORES)), trace=_trace,
    )
    out = np.concatenate([res.results[i]["y"] for i in range(NCORES)], axis=0)
    if _trace:
        kernel._last_results = res
    return out.reshape(B, S, O)
